# revision 38
# baseline (speedup 1.0000x reference)
"""Multi-head causal attention (B=4, S=2048, H=16, d=64, EMB=1024) on 8 trn2 cores.

Sharding: core c handles batch b = c // 2 and head-group g = c % 2
(8 of 16 heads), i.e. a 512-wide slice of the QKV projection dims.

Device kernel (per core):
  - Q^T, K^T projections in [dims, tokens] layout; fp8e4 DoubleRow (weights
    pre-scaled x128 on host, 2^-14 compensation folded into the exp scale)
    or fp16 fallback. V in [tokens, dims] fp16 with a ones-column per head
    (softmax denominator trick).
  - Scores computed transposed: S^T[kv, q]; the two heads of a dim-block
    go into ONE [128,1024] PSUM tile as two concurrent row-tiled matmuls
    (tile_position (0,0)/(64,0)) - measured ~259ns per pair vs 431 serial.
  - exp split across engines: ScalarE ACTIVATE(Exp) for most blocks, DVE
    Schraudolph (i16 = round(s*A + B), bitcast fp16; one tensor_scalar op)
    for a fraction of jq>=1 blocks (rows q>=512, where softmax support is
    large and the +-3% exp approximation error cancels; measured rel err
    1.1e-3 vs the 2e-2 gate).
  - Causal mask: DVE multiply by 0/1 mask on diagonal blocks (mask stored
    duplicated for both heads so one [128,1024] tensor_tensor covers a block).
Host: x transposes + fp8/fp16 casts, weight slicing/transpose (1/sqrt(d)
folded into w_q), final divide-by-denominator + head concat + b_v add.
"""

import os
import sys

import numpy as np

for _p in ("/opt/trn_rl_repo",):
    if _p not in sys.path:
        sys.path.insert(0, _p)

import concourse.bass as bass
import concourse.bacc as bacc
import concourse.mybir as mybir
from concourse.tile import TileContext
from concourse.bass_utils import run_bass_kernel_spmd

try:
    import ml_dtypes
    _F8NP = ml_dtypes.float8_e4m3fn
except Exception:  # pragma: no cover
    _F8NP = None

EMB, QK, V, H = 1024, 64, 64, 16
B, S = 4, 2048
NCORE = 8
HPC = H // 2            # heads per core
DPC = HPC * QK          # projection dims per core (512)
VW = V + 1              # V plus ones-column (65)
NE = EMB // 128         # 8 contraction blocks
ND = DPC // 128         # 4 dim blocks
NQ = S // 512           # 4 q tiles
NT = S // 128           # 16 kv/token blocks
F32 = mybir.dt.float32
F16 = mybir.dt.float16
I16 = mybir.dt.int16
F8 = mybir.dt.float8e4
EXP = mybir.ActivationFunctionType.Exp
MULT = mybir.AluOpType.mult
ADD = mybir.AluOpType.add

FP8_PROJ = False         # fp8e4 DoubleRow Q/K projections: rel err 0.021 >
                         # the 2e-2 gate (e4m3 noise on x and w) - disabled
W8SCALE = 128.0          # pre-scale on w_q/w_k before fp8 cast
SCOMP = 2.0 ** -14 if FP8_PROJ else 1.0   # score compensation (x128 * x128)
SCH_A = 1024.0 / np.log(2.0)              # fp16 schraudolph multiplier
SCH_C = 44.0                              # rel-err-balancing offset
SCH_JQ = 1               # schraudolph only for q-tiles >= this (q >= 512)

_cache = {}
last_results = None


def _build_nc(zero_bias=True):
    nc = bacc.Bacc(None, target_bir_lowering=False)
    x_kT = nc.declare_dram_parameter("x_kT", [EMB, S], F16, isOutput=False)
    w_vT = nc.declare_dram_parameter("w_vT", [EMB, DPC], F16, isOutput=False)
    QKDT = F8 if FP8_PROJ else F16
    x_qT = nc.declare_dram_parameter("x_qT", [EMB, S], QKDT, isOutput=False)
    if FP8_PROJ:
        x_kT8 = nc.declare_dram_parameter("x_kT8", [EMB, S], QKDT, isOutput=False)
    w_qT = nc.declare_dram_parameter("w_qT", [EMB, DPC], QKDT, isOutput=False)
    w_kT = nc.declare_dram_parameter("w_kT", [EMB, DPC], QKDT, isOutput=False)
    b_qk = nc.declare_dram_parameter("b_qk", [128, 2 * ND], F32, isOutput=False)
    consts = nc.declare_dram_parameter("consts", [128, 4 * 1024], F16, isOutput=False)
    z_raw = nc.declare_dram_parameter("z_raw", [HPC, VW, S], F16, isOutput=True)

    with TileContext(nc) as tc:
        with tc.tile_pool(name="const", bufs=1) as cp, \
             tc.tile_pool(name="xk16", bufs=NQ) as xp16, \
             tc.tile_pool(name="x8", bufs=(2 * NQ if FP8_PROJ else NQ)) as xp8, \
             tc.tile_pool(name="pt", bufs=6) as pp, \
             tc.tile_pool(name="zout", bufs=2 * HPC) as zo:
            # persistent SBUF tensors
            wv_sb = cp.tile([128, NE * DPC], F16)
            wq_sb = cp.tile([128, NE * DPC], QKDT)
            wk_sb = cp.tile([128, NE * DPC], QKDT)
            bqk_sb = cp.tile([128, 2 * ND], F32)
            um_sb = cp.tile([128, 4 * 1024], F16)
            QT = cp.tile([128, ND * S], F16)     # [dim-in-dblk, dblk*S + tok]
            KT = cp.tile([128, ND * S], F16)
            VP = cp.tile([128, NT * HPC * VW], F16)  # [tok-in-blk, blk*520 + h*65 + d]

            bq_sb, bk_sb = bqk_sb[:, 0:ND], bqk_sb[:, ND:2 * ND]

            # warm tile memset first: no DMA deps, so the warmup matmuls can
            # run during the input load instead of queueing behind DMA waits
            warm = cp.tile([128, 512], F16)
            nc.vector.memset(warm[:, :], 0.25)

            # ---- DMAs in first-use order ----
            sxk, sxk8, sxq8 = [], [], []

            def dma_xk16(qb):
                t = xp16.tile([128, NE * 512], F16, tag="xk16", name=f"sxk{qb}")
                nc.sync.dma_start(
                    out=t.rearrange("p (e t) -> p e t", e=NE),
                    in_=x_kT[:, qb * 512:(qb + 1) * 512]
                    .rearrange("(e p) t -> p e t", p=128))
                sxk.append(t)

            def dma_x8(lst, src, qb, nm):
                t = xp8.tile([128, NE * 512], QKDT, tag="x8", name=f"{nm}{qb}")
                nc.sync.dma_start(
                    out=t.rearrange("p (e t) -> p e t", e=NE),
                    in_=src[:, qb * 512:(qb + 1) * 512]
                    .rearrange("(e p) t -> p e t", p=128))
                lst.append(t)

            # wv and the first x_k stripe land as 2-e chunks so the first
            # V-projection matmuls can start ~8us earlier (finer splits lose
            # to the ~0.6us per-DMA descriptor-issue cost on the Sync queue)
            t = xp16.tile([128, NE * 512], F16, tag="xk16", name="sxk0")
            for e in range(0, NE, 2):
                nc.sync.dma_start(
                    out=wv_sb.rearrange("p (e d) -> p e d", e=NE)[:, e:e + 2, :],
                    in_=w_vT[e * 128:(e + 2) * 128, :]
                    .rearrange("(e p) d -> p e d", p=128))
                nc.sync.dma_start(
                    out=t.rearrange("p (e t) -> p e t", e=NE)[:, e:e + 2, :],
                    in_=x_kT[e * 128:(e + 2) * 128, 0:512]
                    .rearrange("(e p) t -> p e t", p=128))
            sxk.append(t)
            if FP8_PROJ:
                dma_x8(sxk8, x_kT8, 0, "sxk8_")
            # prologue-critical order: wk, xq0, wq unlock the 8 (dblk, qb=0)
            # projection chunks; um/bqk are not needed until attention starts
            nc.sync.dma_start(
                out=wk_sb.rearrange("p (e d) -> p e d", e=NE),
                in_=w_kT.rearrange("(e p) d -> p e d", p=128))
            dma_x8(sxq8, x_qT, 0, "sxq8_")
            nc.sync.dma_start(
                out=wq_sb.rearrange("p (e d) -> p e d", e=NE),
                in_=w_qT.rearrange("(e p) d -> p e d", p=128))
            nc.sync.dma_start(out=bqk_sb[:, :], in_=b_qk[:, :])
            nc.sync.dma_start(out=um_sb[:, :], in_=consts[:, :])
            for qb in range(1, NQ):
                dma_xk16(qb)
                if FP8_PROJ:
                    dma_x8(sxk8, x_kT8, qb, "sxk8_")
                dma_x8(sxq8, x_qT, qb, "sxq8_")
            if not FP8_PROJ:
                sxk8 = sxk    # K projection reads the fp16 x_k stripes

            # ones columns for the denominator trick (V copies fill cols 0-63;
            # only col 64 of each head-block needs the 1.0 fill)
            nc.vector.memset(
                VP.rearrange("p (t w) -> p t w", w=VW)[:, :, V:VW], 1.0)
            # pre-warm DVE's vector clock on the const DMAs so later DVE ops
            # don't each carry DMA-sem waits (walrus wait-slot limits)
            scr = cp.tile([128, 2], F32)
            scrh = cp.tile([128, 1], F16)
            nc.vector.tensor_copy(scr[:, 0:1], wv_sb[:, 0:2].bitcast(F32))
            nc.vector.tensor_copy(scrh[:, 0:1], sxk[0][:, 0:1])
            # pre-warm PE's clock too (dummy weight loads): fused LW+MM pairs
            # have a ~2-slot combined sync-wait budget in walrus codegen, so
            # absorb the const-DMA and DVE deps before real matmuls start
            for ap in (wq_sb, wk_sb, wv_sb, um_sb, scrh):
                nc.tensor.ldweights(ap[0:64, 0:1])

            with tc.tile_pool(name="pj", bufs=2, space="PSUM") as pj:
                wps = pj.tile([128, 512], F32, tag="big", bufs=3, name="warmps")
                for _ in range(16):
                    nc.tensor.matmul(wps[:, :], lhsT=warm[:, 0:128],
                                     rhs=warm[:, :], start=True, stop=True,
                                     skip_group_check=True)

                # V[t, d] with ones column; feeds the attention stream
                def proj_v(tb):
                    qb, t = divmod(tb, 4)
                    ps = pj.tile([128, 512], F32, tag="big", bufs=3, name=f"pv{tb}")
                    for e in range(NE):
                        nc.tensor.matmul(
                            ps[:, :],
                            lhsT=sxk[qb][:, e * 512 + t * 128: e * 512 + (t + 1) * 128],
                            rhs=wv_sb[:, e * DPC:(e + 1) * DPC],
                            start=(e == 0), stop=(e == NE - 1))
                    dst = VP[:, tb * (HPC * VW):(tb + 1) * (HPC * VW)]
                    dst = dst.rearrange("p (h w) -> p h w", w=VW)[:, :, 0:V]
                    nc.vector.tensor_copy(
                        dst, ps[:, :].rearrange("p (h w) -> p h w", w=V))

                # K^T / Q^T chunk for one (dblk, qb)
                def proj_kq(which, dblk, qb):
                    wsb, bsb, OUT, sx = ((wk_sb, bk_sb, KT, sxk8) if which == "k"
                                         else (wq_sb, bq_sb, QT, sxq8))
                    ps = pj.tile([128, 512], F32, tag="big", bufs=3,
                                 name=f"p{which}{dblk}{qb}")
                    if FP8_PROJ:
                        w3 = wsb.rearrange("p (e d) -> p e d", e=NE)
                        x3 = sx[qb].rearrange("p (e t) -> p e t", e=NE)
                        for ep in range(NE // 2):
                            nc.tensor.matmul(
                                ps[:, :],
                                lhsT=w3[:, 2 * ep:2 * ep + 2,
                                        dblk * 128:(dblk + 1) * 128],
                                rhs=x3[:, 2 * ep:2 * ep + 2, :],
                                start=(ep == 0), stop=(ep == NE // 2 - 1),
                                perf_mode=mybir.MatmulPerfMode.DoubleRow)
                    else:
                        for e in range(NE):
                            nc.tensor.matmul(
                                ps[:, :],
                                lhsT=wsb[:, e * DPC + dblk * 128:
                                         e * DPC + (dblk + 1) * 128],
                                rhs=sx[qb][:, e * 512:(e + 1) * 512],
                                start=(e == 0), stop=(e == NE - 1))
                    dst = OUT[:, dblk * S + qb * 512: dblk * S + (qb + 1) * 512]
                    if zero_bias:
                        # ScalarE copy: frees DVE time and releases the PSUM
                        # slot sooner (DVE queue is the busier one)
                        nc.scalar.copy(dst, ps[:, :])
                    else:
                        nc.vector.tensor_scalar_add(dst, ps[:, :],
                                                    bsb[:, dblk:dblk + 1])

                # prologue: everything computable from the early DMAs
                # (xk stripe 0, wk, xq stripe 0, wq) - all dblks' qb=0
                # chunks, so the PE has ~28us of work while inputs stream in
                for tb in range(4):
                    proj_v(tb)
                for d in range(ND):
                    proj_kq("k", d, 0)
                    proj_kq("q", d, 0)

                # attention for head pair (2*dblk, 2*dblk+1); both heads'
                # scores land in ONE [128,1024] PSUM tile via two concurrent
                # row-tiled matmuls, so exp handles both heads in one instr
                def attention_pair(dblk, feed):
                    heads = (2 * dblk, 2 * dblk + 1)

                    # pending = (pts of one g, zps, nkv, jq, last-g?) issued
                    # one g later so exp/mask have a full iteration of slack
                    # before PE consumes pts - carried across jq boundaries
                    pending = [None]

                    def issue_pv(pend):
                        cur, zps_, nkv_, jq_, last = pend
                        for pt, i in cur:
                            for hi in (0, 1):
                                nc.tensor.matmul(
                                    zps_[hi][:, :],
                                    lhsT=VP[:, i * (HPC * VW) + heads[hi] * VW:
                                            i * (HPC * VW) + (heads[hi] + 1) * VW],
                                    rhs=pt[:, hi * 512:(hi + 1) * 512],
                                    start=(i == 0), stop=(i == nkv_ - 1),
                                    skip_group_check=True)
                        if last:
                            for hi in (0, 1):
                                zsb = zo.tile([VW, 512], F16, tag="zsb",
                                              name=f"zsb{heads[hi]}_{jq_}")
                                nc.scalar.copy(zsb[:, :], zps_[hi][:, :])
                                nc.sync.dma_start(
                                    out=z_raw[heads[hi], :,
                                              jq_ * 512:(jq_ + 1) * 512],
                                    in_=zsb[:, :])

                    for jq in range(NQ):
                        nkv = 4 * (jq + 1)
                        qs = slice(dblk * S + jq * 512, dblk * S + (jq + 1) * 512)
                        zps = [pj.tile([VW, 512], F32, tag="zps", bufs=2,
                                       name=f"z{hi}_{jq % 2}") for hi in (0, 1)]
                        for g in range(nkv // 2):
                            for _ in range(2):
                                if feed:
                                    feed.pop(0)()
                            cur = []
                            for bs in range(2):
                                i = 2 * g + bs
                                kv = slice(dblk * S + i * 128,
                                           dblk * S + (i + 1) * 128)
                                sp = pj.tile([128, 1024], F32, tag="big",
                                             bufs=3, name=f"sp{bs}")
                                nc.tensor.matmul(
                                    sp[:, 0:512], lhsT=KT[0:64, kv],
                                    rhs=QT[0:64, qs], start=True, stop=True,
                                    tile_position=(0, 0))
                                nc.tensor.matmul(
                                    sp[:, 512:1024], lhsT=KT[64:128, kv],
                                    rhs=QT[64:128, qs], start=True, stop=True,
                                    tile_position=(64, 0))
                                pt = pp.tile([128, 1024], F16, tag="pt",
                                             name=f"pt{bs}")
                                # per-g 1:1 ACT/DVE split keeps both engines
                                # under PE's per-g budget in feed-less phases
                                use_dve = (jq >= SCH_JQ and bs == 1)
                                if use_dve:
                                    nc.vector.tensor_scalar(
                                        pt[:, :].bitcast(I16), sp[:, :],
                                        float(SCH_A * SCOMP),
                                        float(15 * 1024 - SCH_C), MULT, ADD)
                                else:
                                    nc.scalar.activation(
                                        pt[:, :], sp[:, :], EXP,
                                        scale=float(SCOMP))
                                if i >= 4 * jq:      # diagonal: zero the
                                    bb = i - 4 * jq  # upper triangle on DVE
                                    nc.vector.tensor_mul(
                                        pt[:, :], pt[:, :],
                                        um_sb[:, bb * 1024:(bb + 1) * 1024])
                                cur.append((pt, i))
                            if pending[0]:
                                issue_pv(pending[0])
                            pending[0] = (cur, zps, nkv, jq,
                                          g == nkv // 2 - 1)
                    issue_pv(pending[0])

                for dblk in range(ND):
                    feed = []
                    if dblk == 0:
                        for q in (1, 2, 3):
                            feed += [lambda q=q: proj_kq("k", 0, q),
                                     lambda q=q: proj_kq("q", 0, q)]
                            feed += [(lambda tb=tb: proj_v(tb))
                                     for tb in range(4 * q, 4 * q + 4)]
                    if dblk + 1 < ND:
                        feed += [(lambda w=w, d=dblk + 1, q=q: proj_kq(w, d, q))
                                 for q in range(1, NQ) for w in ("k", "q")]
                    attention_pair(dblk, feed)
                    for f in feed:
                        f()

    nc.compile()
    return nc


def kernel(x_q, x_k_v, attn_mask, w_q, b_q, w_k, b_k, w_v, b_v):
    global last_results
    x_q = np.ascontiguousarray(x_q, np.float32)
    x_k_v = np.ascontiguousarray(x_k_v, np.float32)
    w_q, w_k, w_v = (np.asarray(a, np.float32) for a in (w_q, w_k, w_v))
    b_q, b_k, b_v = (np.asarray(a, np.float32) for a in (b_q, b_k, b_v))

    zero_bias = not (np.any(b_q) or np.any(b_k))
    key = f"nc{zero_bias}"
    if key not in _cache:
        _cache[key] = _build_nc(zero_bias)
    nc = _cache[key]

    scale = 1.0 / np.sqrt(np.float32(QK))
    qkdt = _F8NP if FP8_PROJ else np.float16
    wsc = W8SCALE if FP8_PROJ else 1.0
    xkT16 = [np.ascontiguousarray(x_k_v[b].T).astype(np.float16) for b in range(B)]
    xqT8 = [np.ascontiguousarray(x_q[b].T).astype(qkdt) for b in range(B)]
    xkT8 = ([np.ascontiguousarray(x_k_v[b].T).astype(qkdt) for b in range(B)]
            if FP8_PROJ else None)
    wqT = [np.ascontiguousarray((w_q[g * DPC:(g + 1) * DPC] * (scale * wsc)).T)
           .astype(qkdt) for g in range(2)]
    wkT = [np.ascontiguousarray((w_k[g * DPC:(g + 1) * DPC] * wsc).T)
           .astype(qkdt) for g in range(2)]
    wvT = [np.ascontiguousarray(w_v[g * DPC:(g + 1) * DPC].T).astype(np.float16)
           for g in range(2)]
    bq2 = [np.ascontiguousarray(
        (b_q[g * DPC:(g + 1) * DPC] * (scale * wsc * wsc)).reshape(ND, 128).T)
        for g in range(2)]
    bk2 = [np.ascontiguousarray(
        (b_k[g * DPC:(g + 1) * DPC] * (wsc * wsc)).reshape(ND, 128).T)
        for g in range(2)]
    # 0/1 causal keep-masks for the 4 diagonal 128x512 blocks, duplicated for
    # the two heads packed side-by-side in one [128,1024] probability tile
    p = np.arange(128)[:, None]
    qq = np.arange(512)[None, :]
    um = np.concatenate(
        [np.tile(np.where(128 * bb + p > qq, np.float32(0.0), np.float32(1.0)),
                 (1, 2))
         for bb in range(4)], axis=1).astype(np.float16)
    bqk2 = [np.ascontiguousarray(np.concatenate([bq2[g], bk2[g]], axis=1))
            for g in range(2)]

    in_maps = []
    for c in range(NCORE):
        b, g = c // 2, c % 2
        m = {
            "x_qT": xqT8[b], "x_kT": xkT16[b],
            "w_qT": wqT[g], "w_kT": wkT[g], "w_vT": wvT[g],
            "b_qk": bqk2[g], "consts": um,
        }
        if FP8_PROJ:
            m["x_kT8"] = xkT8[b]
        in_maps.append(m)

    trace = os.environ.get("KERNEL_TRACE", "") == "1"
    res = run_bass_kernel_spmd(nc, in_maps, list(range(NCORE)), trace=trace)
    last_results = res

    out = np.empty((B, S, H * V), np.float32)
    for c in range(NCORE):
        b, g = c // 2, c % 2
        zr = res.results[c]["z_raw"].astype(np.float32)   # [HPC, VW, S]
        z = zr[:, :V, :] / zr[:, V:VW, :]                  # [HPC, V, S]
        out[b, :, g * DPC:(g + 1) * DPC] = z.transpose(2, 0, 1).reshape(S, DPC)
    out += b_v[None, None, :]
    return out


# revision 40
# speedup vs baseline: 1.0204x; 1.0204x over previous
"""Multi-head causal attention (B=4, S=2048, H=16, d=64, EMB=1024) on 8 trn2 cores.

Sharding: core c handles batch b = c // 2 and head-group g = c % 2
(8 of 16 heads), i.e. a 512-wide slice of the QKV projection dims.

Device kernel (per core):
  - Q^T, K^T projections in [dims, tokens] layout; fp8e4 DoubleRow (weights
    pre-scaled x128 on host, 2^-14 compensation folded into the exp scale)
    or fp16 fallback. V in [tokens, dims] fp16 with a ones-column per head
    (softmax denominator trick).
  - Scores computed transposed: S^T[kv, q]; the two heads of a dim-block
    go into ONE [128,1024] PSUM tile as two concurrent row-tiled matmuls
    (tile_position (0,0)/(64,0)) - measured ~259ns per pair vs 431 serial.
  - exp split across engines: ScalarE ACTIVATE(Exp) for most blocks, DVE
    Schraudolph (i16 = round(s*A + B), bitcast fp16; one tensor_scalar op)
    for a fraction of jq>=1 blocks (rows q>=512, where softmax support is
    large and the +-3% exp approximation error cancels; measured rel err
    1.1e-3 vs the 2e-2 gate).
  - Causal mask: DVE multiply by 0/1 mask on diagonal blocks (mask stored
    duplicated for both heads so one [128,1024] tensor_tensor covers a block).
Host: x transposes + fp8/fp16 casts, weight slicing/transpose (1/sqrt(d)
folded into w_q), final divide-by-denominator + head concat + b_v add.
"""

import os
import sys

import numpy as np

for _p in ("/opt/trn_rl_repo",):
    if _p not in sys.path:
        sys.path.insert(0, _p)

import concourse.bass as bass
import concourse.bacc as bacc
import concourse.mybir as mybir
from concourse.tile import TileContext
from concourse.bass_utils import run_bass_kernel_spmd

try:
    import ml_dtypes
    _F8NP = ml_dtypes.float8_e4m3fn
except Exception:  # pragma: no cover
    _F8NP = None

EMB, QK, V, H = 1024, 64, 64, 16
B, S = 4, 2048
NCORE = 8
HPC = H // 2            # heads per core
DPC = HPC * QK          # projection dims per core (512)
VW = V + 1              # V plus ones-column (65)
NE = EMB // 128         # 8 contraction blocks
ND = DPC // 128         # 4 dim blocks
NQ = S // 512           # 4 q tiles
NT = S // 128           # 16 kv/token blocks
F32 = mybir.dt.float32
F16 = mybir.dt.float16
I16 = mybir.dt.int16
F8 = mybir.dt.float8e4
EXP = mybir.ActivationFunctionType.Exp
MULT = mybir.AluOpType.mult
ADD = mybir.AluOpType.add

FP8_PROJ = False         # fp8e4 DoubleRow Q/K projections: rel err 0.021 >
                         # the 2e-2 gate (e4m3 noise on x and w) - disabled
W8SCALE = 128.0          # pre-scale on w_q/w_k before fp8 cast
SCOMP = 2.0 ** -14 if FP8_PROJ else 1.0   # score compensation (x128 * x128)
SCH_A = 1024.0 / np.log(2.0)              # fp16 schraudolph multiplier
SCH_C = 44.0                              # rel-err-balancing offset
SCH_JQ = 1               # schraudolph only for q-tiles >= this (q >= 512)

_cache = {}
last_results = None


def _build_nc(zero_bias=True):
    nc = bacc.Bacc(None, target_bir_lowering=False)
    x_kT = nc.declare_dram_parameter("x_kT", [EMB, S], F16, isOutput=False)
    w_vT = nc.declare_dram_parameter("w_vT", [EMB, DPC], F16, isOutput=False)
    QKDT = F8 if FP8_PROJ else F16
    x_qT = nc.declare_dram_parameter("x_qT", [EMB, S], QKDT, isOutput=False)
    if FP8_PROJ:
        x_kT8 = nc.declare_dram_parameter("x_kT8", [EMB, S], QKDT, isOutput=False)
    w_qT = nc.declare_dram_parameter("w_qT", [EMB, DPC], QKDT, isOutput=False)
    w_kT = nc.declare_dram_parameter("w_kT", [EMB, DPC], QKDT, isOutput=False)
    b_qk = nc.declare_dram_parameter("b_qk", [128, 2 * ND], F32, isOutput=False)
    consts = nc.declare_dram_parameter("consts", [128, 4 * 1024], F16, isOutput=False)
    z_raw = nc.declare_dram_parameter("z_raw", [HPC, VW, S], F16, isOutput=True)

    with TileContext(nc) as tc:
        with tc.tile_pool(name="const", bufs=1) as cp, \
             tc.tile_pool(name="xk16", bufs=NQ) as xp16, \
             tc.tile_pool(name="x8", bufs=(2 * NQ if FP8_PROJ else NQ)) as xp8, \
             tc.tile_pool(name="pt", bufs=6) as pp, \
             tc.tile_pool(name="zout", bufs=2 * HPC) as zo:
            # persistent SBUF tensors
            wv_sb = cp.tile([128, NE * DPC], F16)
            wq_sb = cp.tile([128, NE * DPC], QKDT)
            wk_sb = cp.tile([128, NE * DPC], QKDT)
            bqk_sb = cp.tile([128, 2 * ND], F32)
            um_sb = cp.tile([128, 4 * 1024], F16)
            QT = cp.tile([128, ND * S], F16)     # [dim-in-dblk, dblk*S + tok]
            KT = cp.tile([128, ND * S], F16)
            VP = cp.tile([128, NT * HPC * VW], F16)  # [tok-in-blk, blk*520 + h*65 + d]

            bq_sb, bk_sb = bqk_sb[:, 0:ND], bqk_sb[:, ND:2 * ND]

            # warm tile memset first: no DMA deps, so the warmup matmuls can
            # run during the input load instead of queueing behind DMA waits
            warm = cp.tile([128, 512], F16)
            nc.vector.memset(warm[:, :], 0.25)

            # ---- DMAs in first-use order ----
            sxk, sxk8, sxq8 = [], [], []

            def dma_xk16(qb):
                t = xp16.tile([128, NE * 512], F16, tag="xk16", name=f"sxk{qb}")
                nc.sync.dma_start(
                    out=t.rearrange("p (e t) -> p e t", e=NE),
                    in_=x_kT[:, qb * 512:(qb + 1) * 512]
                    .rearrange("(e p) t -> p e t", p=128))
                sxk.append(t)

            def dma_x8(lst, src, qb, nm):
                t = xp8.tile([128, NE * 512], QKDT, tag="x8", name=f"{nm}{qb}")
                nc.sync.dma_start(
                    out=t.rearrange("p (e t) -> p e t", e=NE),
                    in_=src[:, qb * 512:(qb + 1) * 512]
                    .rearrange("(e p) t -> p e t", p=128))
                lst.append(t)

            # wv and the first x_k stripe land as 2-e chunks so the first
            # V-projection matmuls can start ~8us earlier (finer splits lose
            # to the ~0.6us per-DMA descriptor-issue cost on the Sync queue)
            t = xp16.tile([128, NE * 512], F16, tag="xk16", name="sxk0")
            for e in range(0, NE, 2):
                nc.sync.dma_start(
                    out=wv_sb.rearrange("p (e d) -> p e d", e=NE)[:, e:e + 2, :],
                    in_=w_vT[e * 128:(e + 2) * 128, :]
                    .rearrange("(e p) d -> p e d", p=128))
                nc.sync.dma_start(
                    out=t.rearrange("p (e t) -> p e t", e=NE)[:, e:e + 2, :],
                    in_=x_kT[e * 128:(e + 2) * 128, 0:512]
                    .rearrange("(e p) t -> p e t", p=128))
            sxk.append(t)
            # um/bqk early: the DVE pre-warm copies below wait on these, and
            # they gate the whole DVE queue (first V-proj casts included)
            nc.sync.dma_start(out=bqk_sb[:, :], in_=b_qk[:, :])
            nc.sync.dma_start(out=um_sb[:, :], in_=consts[:, :])
            if FP8_PROJ:
                dma_x8(sxk8, x_kT8, 0, "sxk8_")
            nc.sync.dma_start(
                out=wk_sb.rearrange("p (e d) -> p e d", e=NE),
                in_=w_kT.rearrange("(e p) d -> p e d", p=128))
            dma_x8(sxq8, x_qT, 0, "sxq8_")
            nc.sync.dma_start(
                out=wq_sb.rearrange("p (e d) -> p e d", e=NE),
                in_=w_qT.rearrange("(e p) d -> p e d", p=128))
            for qb in range(1, NQ):
                dma_xk16(qb)
                if FP8_PROJ:
                    dma_x8(sxk8, x_kT8, qb, "sxk8_")
                dma_x8(sxq8, x_qT, qb, "sxq8_")
            if not FP8_PROJ:
                sxk8 = sxk    # K projection reads the fp16 x_k stripes

            # ones columns for the denominator trick (V copies fill cols 0-63;
            # only col 64 of each head-block needs the 1.0 fill)
            nc.vector.memset(
                VP.rearrange("p (t w) -> p t w", w=VW)[:, :, V:VW], 1.0)
            # pre-warm DVE's vector clock on the const DMAs so later DVE ops
            # don't each carry DMA-sem waits (walrus wait-slot limits)
            scr = cp.tile([128, 2], F32)
            scrh = cp.tile([128, 1], F16)
            nc.vector.tensor_copy(scr[:, 0:1], bqk_sb[:, 0:1])
            nc.vector.tensor_copy(scrh[:, 0:1], um_sb[:, 0:1])
            # pre-warm PE's clock too (dummy weight loads): fused LW+MM pairs
            # have a ~2-slot combined sync-wait budget in walrus codegen, so
            # absorb the const-DMA and DVE deps before real matmuls start
            for ap in (wq_sb, wk_sb, wv_sb, um_sb, scrh):
                nc.tensor.ldweights(ap[0:64, 0:1])

            with tc.tile_pool(name="pj", bufs=2, space="PSUM") as pj:
                wps = pj.tile([128, 512], F32, tag="big", bufs=3, name="warmps")
                for _ in range(16):
                    nc.tensor.matmul(wps[:, :], lhsT=warm[:, 0:128],
                                     rhs=warm[:, :], start=True, stop=True,
                                     skip_group_check=True)

                # V[t, d] with ones column; feeds the attention stream
                def proj_v(tb):
                    qb, t = divmod(tb, 4)
                    ps = pj.tile([128, 512], F32, tag="big", bufs=3, name=f"pv{tb}")
                    for e in range(NE):
                        nc.tensor.matmul(
                            ps[:, :],
                            lhsT=sxk[qb][:, e * 512 + t * 128: e * 512 + (t + 1) * 128],
                            rhs=wv_sb[:, e * DPC:(e + 1) * DPC],
                            start=(e == 0), stop=(e == NE - 1))
                    dst = VP[:, tb * (HPC * VW):(tb + 1) * (HPC * VW)]
                    dst = dst.rearrange("p (h w) -> p h w", w=VW)[:, :, 0:V]
                    nc.vector.tensor_copy(
                        dst, ps[:, :].rearrange("p (h w) -> p h w", w=V))

                # K^T / Q^T chunk for one (dblk, qb)
                def proj_kq(which, dblk, qb):
                    wsb, bsb, OUT, sx = ((wk_sb, bk_sb, KT, sxk8) if which == "k"
                                         else (wq_sb, bq_sb, QT, sxq8))
                    ps = pj.tile([128, 512], F32, tag="big", bufs=3,
                                 name=f"p{which}{dblk}{qb}")
                    if FP8_PROJ:
                        w3 = wsb.rearrange("p (e d) -> p e d", e=NE)
                        x3 = sx[qb].rearrange("p (e t) -> p e t", e=NE)
                        for ep in range(NE // 2):
                            nc.tensor.matmul(
                                ps[:, :],
                                lhsT=w3[:, 2 * ep:2 * ep + 2,
                                        dblk * 128:(dblk + 1) * 128],
                                rhs=x3[:, 2 * ep:2 * ep + 2, :],
                                start=(ep == 0), stop=(ep == NE // 2 - 1),
                                perf_mode=mybir.MatmulPerfMode.DoubleRow)
                    else:
                        for e in range(NE):
                            nc.tensor.matmul(
                                ps[:, :],
                                lhsT=wsb[:, e * DPC + dblk * 128:
                                         e * DPC + (dblk + 1) * 128],
                                rhs=sx[qb][:, e * 512:(e + 1) * 512],
                                start=(e == 0), stop=(e == NE - 1))
                    dst = OUT[:, dblk * S + qb * 512: dblk * S + (qb + 1) * 512]
                    if zero_bias:
                        # ScalarE copy: frees DVE time and releases the PSUM
                        # slot sooner (DVE queue is the busier one)
                        nc.scalar.copy(dst, ps[:, :])
                    else:
                        nc.vector.tensor_scalar_add(dst, ps[:, :],
                                                    bsb[:, dblk:dblk + 1])

                # prologue: everything computable from the early DMAs
                # (xk stripe 0, wk, xq stripe 0, wq) - all dblks' qb=0
                # chunks, so the PE has ~28us of work while inputs stream in
                for tb in range(4):
                    proj_v(tb)
                for d in range(ND):
                    proj_kq("k", d, 0)
                    proj_kq("q", d, 0)

                # attention for head pair (2*dblk, 2*dblk+1); both heads'
                # scores land in ONE [128,1024] PSUM tile via two concurrent
                # row-tiled matmuls, so exp handles both heads in one instr
                def attention_pair(dblk, feed):
                    heads = (2 * dblk, 2 * dblk + 1)

                    # pending = (pts of one g, zps, nkv, jq, last-g?) issued
                    # one g later so exp/mask have a full iteration of slack
                    # before PE consumes pts - carried across jq boundaries
                    pending = [None]

                    def issue_pv(pend):
                        cur, zps_, nkv_, jq_, last = pend
                        for pt, i in cur:
                            for hi in (0, 1):
                                nc.tensor.matmul(
                                    zps_[hi][:, :],
                                    lhsT=VP[:, i * (HPC * VW) + heads[hi] * VW:
                                            i * (HPC * VW) + (heads[hi] + 1) * VW],
                                    rhs=pt[:, hi * 512:(hi + 1) * 512],
                                    start=(i == 0), stop=(i == nkv_ - 1),
                                    skip_group_check=True)
                        if last:
                            for hi in (0, 1):
                                zsb = zo.tile([VW, 512], F16, tag="zsb",
                                              name=f"zsb{heads[hi]}_{jq_}")
                                nc.scalar.copy(zsb[:, :], zps_[hi][:, :])
                                nc.sync.dma_start(
                                    out=z_raw[heads[hi], :,
                                              jq_ * 512:(jq_ + 1) * 512],
                                    in_=zsb[:, :])

                    for jq in range(NQ):
                        nkv = 4 * (jq + 1)
                        qs = slice(dblk * S + jq * 512, dblk * S + (jq + 1) * 512)
                        zps = [pj.tile([VW, 512], F32, tag="zps", bufs=2,
                                       name=f"z{hi}_{jq % 2}") for hi in (0, 1)]
                        for g in range(nkv // 2):
                            for _ in range(2):
                                if feed:
                                    feed.pop(0)()
                            cur = []
                            for bs in range(2):
                                i = 2 * g + bs
                                kv = slice(dblk * S + i * 128,
                                           dblk * S + (i + 1) * 128)
                                sp = pj.tile([128, 1024], F32, tag="big",
                                             bufs=3, name=f"sp{bs}")
                                nc.tensor.matmul(
                                    sp[:, 0:512], lhsT=KT[0:64, kv],
                                    rhs=QT[0:64, qs], start=True, stop=True,
                                    tile_position=(0, 0))
                                nc.tensor.matmul(
                                    sp[:, 512:1024], lhsT=KT[64:128, kv],
                                    rhs=QT[64:128, qs], start=True, stop=True,
                                    tile_position=(64, 0))
                                pt = pp.tile([128, 1024], F16, tag="pt",
                                             name=f"pt{bs}")
                                # per-g 1:1 ACT/DVE split keeps both engines
                                # under PE's per-g budget in feed-less phases
                                use_dve = (jq >= SCH_JQ and bs == 1)
                                if use_dve:
                                    nc.vector.tensor_scalar(
                                        pt[:, :].bitcast(I16), sp[:, :],
                                        float(SCH_A * SCOMP),
                                        float(15 * 1024 - SCH_C), MULT, ADD)
                                else:
                                    nc.scalar.activation(
                                        pt[:, :], sp[:, :], EXP,
                                        scale=float(SCOMP))
                                if i >= 4 * jq:      # diagonal: zero the
                                    bb = i - 4 * jq  # upper triangle on DVE
                                    nc.vector.tensor_mul(
                                        pt[:, :], pt[:, :],
                                        um_sb[:, bb * 1024:(bb + 1) * 1024])
                                cur.append((pt, i))
                            if pending[0]:
                                issue_pv(pending[0])
                            pending[0] = (cur, zps, nkv, jq,
                                          g == nkv // 2 - 1)
                    issue_pv(pending[0])

                for dblk in range(ND):
                    feed = []
                    if dblk == 0:
                        for q in (1, 2, 3):
                            feed += [lambda q=q: proj_kq("k", 0, q),
                                     lambda q=q: proj_kq("q", 0, q)]
                            feed += [(lambda tb=tb: proj_v(tb))
                                     for tb in range(4 * q, 4 * q + 4)]
                    if dblk + 1 < ND:
                        feed += [(lambda w=w, d=dblk + 1, q=q: proj_kq(w, d, q))
                                 for q in range(1, NQ) for w in ("k", "q")]
                    attention_pair(dblk, feed)
                    for f in feed:
                        f()

    nc.compile()
    return nc


def kernel(x_q, x_k_v, attn_mask, w_q, b_q, w_k, b_k, w_v, b_v):
    global last_results
    x_q = np.ascontiguousarray(x_q, np.float32)
    x_k_v = np.ascontiguousarray(x_k_v, np.float32)
    w_q, w_k, w_v = (np.asarray(a, np.float32) for a in (w_q, w_k, w_v))
    b_q, b_k, b_v = (np.asarray(a, np.float32) for a in (b_q, b_k, b_v))

    zero_bias = not (np.any(b_q) or np.any(b_k))
    key = f"nc{zero_bias}"
    if key not in _cache:
        _cache[key] = _build_nc(zero_bias)
    nc = _cache[key]

    scale = 1.0 / np.sqrt(np.float32(QK))
    qkdt = _F8NP if FP8_PROJ else np.float16
    wsc = W8SCALE if FP8_PROJ else 1.0
    xkT16 = [np.ascontiguousarray(x_k_v[b].T).astype(np.float16) for b in range(B)]
    xqT8 = [np.ascontiguousarray(x_q[b].T).astype(qkdt) for b in range(B)]
    xkT8 = ([np.ascontiguousarray(x_k_v[b].T).astype(qkdt) for b in range(B)]
            if FP8_PROJ else None)
    wqT = [np.ascontiguousarray((w_q[g * DPC:(g + 1) * DPC] * (scale * wsc)).T)
           .astype(qkdt) for g in range(2)]
    wkT = [np.ascontiguousarray((w_k[g * DPC:(g + 1) * DPC] * wsc).T)
           .astype(qkdt) for g in range(2)]
    wvT = [np.ascontiguousarray(w_v[g * DPC:(g + 1) * DPC].T).astype(np.float16)
           for g in range(2)]
    bq2 = [np.ascontiguousarray(
        (b_q[g * DPC:(g + 1) * DPC] * (scale * wsc * wsc)).reshape(ND, 128).T)
        for g in range(2)]
    bk2 = [np.ascontiguousarray(
        (b_k[g * DPC:(g + 1) * DPC] * (wsc * wsc)).reshape(ND, 128).T)
        for g in range(2)]
    # 0/1 causal keep-masks for the 4 diagonal 128x512 blocks, duplicated for
    # the two heads packed side-by-side in one [128,1024] probability tile
    p = np.arange(128)[:, None]
    qq = np.arange(512)[None, :]
    um = np.concatenate(
        [np.tile(np.where(128 * bb + p > qq, np.float32(0.0), np.float32(1.0)),
                 (1, 2))
         for bb in range(4)], axis=1).astype(np.float16)
    bqk2 = [np.ascontiguousarray(np.concatenate([bq2[g], bk2[g]], axis=1))
            for g in range(2)]

    in_maps = []
    for c in range(NCORE):
        b, g = c // 2, c % 2
        m = {
            "x_qT": xqT8[b], "x_kT": xkT16[b],
            "w_qT": wqT[g], "w_kT": wkT[g], "w_vT": wvT[g],
            "b_qk": bqk2[g], "consts": um,
        }
        if FP8_PROJ:
            m["x_kT8"] = xkT8[b]
        in_maps.append(m)

    trace = os.environ.get("KERNEL_TRACE", "") == "1"
    res = run_bass_kernel_spmd(nc, in_maps, list(range(NCORE)), trace=trace)
    last_results = res

    out = np.empty((B, S, H * V), np.float32)
    for c in range(NCORE):
        b, g = c // 2, c % 2
        zr = res.results[c]["z_raw"].astype(np.float32)   # [HPC, VW, S]
        z = zr[:, :V, :] / zr[:, V:VW, :]                  # [HPC, V, S]
        out[b, :, g * DPC:(g + 1) * DPC] = z.transpose(2, 0, 1).reshape(S, DPC)
    out += b_v[None, None, :]
    return out


# revision 44
# speedup vs baseline: 1.0239x; 1.0035x over previous
"""Multi-head causal attention (B=4, S=2048, H=16, d=64, EMB=1024) on 8 trn2 cores.

Sharding: core c handles batch b = c // 2 and head-group g = c % 2
(8 of 16 heads), i.e. a 512-wide slice of the QKV projection dims.

Device kernel (per core):
  - Q^T, K^T projections in [dims, tokens] layout; fp8e4 DoubleRow (weights
    pre-scaled x128 on host, 2^-14 compensation folded into the exp scale)
    or fp16 fallback. V in [tokens, dims] fp16 with a ones-column per head
    (softmax denominator trick).
  - Scores computed transposed: S^T[kv, q]; the two heads of a dim-block
    go into ONE [128,1024] PSUM tile as two concurrent row-tiled matmuls
    (tile_position (0,0)/(64,0)) - measured ~259ns per pair vs 431 serial.
  - exp split across engines: ScalarE ACTIVATE(Exp) for most blocks, DVE
    Schraudolph (i16 = round(s*A + B), bitcast fp16; one tensor_scalar op)
    for a fraction of jq>=1 blocks (rows q>=512, where softmax support is
    large and the +-3% exp approximation error cancels; measured rel err
    1.1e-3 vs the 2e-2 gate).
  - Causal mask: DVE multiply by 0/1 mask on diagonal blocks (mask stored
    duplicated for both heads so one [128,1024] tensor_tensor covers a block).
Host: x transposes + fp8/fp16 casts, weight slicing/transpose (1/sqrt(d)
folded into w_q), final divide-by-denominator + head concat + b_v add.
"""

import os
import sys

import numpy as np

for _p in ("/opt/trn_rl_repo",):
    if _p not in sys.path:
        sys.path.insert(0, _p)

import concourse.bass as bass
import concourse.bacc as bacc
import concourse.mybir as mybir
from concourse.tile import TileContext
from concourse.bass_utils import run_bass_kernel_spmd

try:
    import ml_dtypes
    _F8NP = ml_dtypes.float8_e4m3fn
except Exception:  # pragma: no cover
    _F8NP = None

EMB, QK, V, H = 1024, 64, 64, 16
B, S = 4, 2048
NCORE = 8
HPC = H // 2            # heads per core
DPC = HPC * QK          # projection dims per core (512)
VW = V + 1              # V plus ones-column (65)
NE = EMB // 128         # 8 contraction blocks
ND = DPC // 128         # 4 dim blocks
NQ = S // 512           # 4 q tiles
NT = S // 128           # 16 kv/token blocks
F32 = mybir.dt.float32
F16 = mybir.dt.float16
I16 = mybir.dt.int16
F8 = mybir.dt.float8e4
EXP = mybir.ActivationFunctionType.Exp
MULT = mybir.AluOpType.mult
ADD = mybir.AluOpType.add

FP8_PROJ = False         # fp8e4 DoubleRow Q/K projections: rel err 0.021 >
                         # the 2e-2 gate (e4m3 noise on x and w) - disabled
W8SCALE = 128.0          # pre-scale on w_q/w_k before fp8 cast
SCOMP = 2.0 ** -14 if FP8_PROJ else 1.0   # score compensation (x128 * x128)
SCH_A = 1024.0 / np.log(2.0)              # fp16 schraudolph multiplier
SCH_C = 44.0                              # rel-err-balancing offset
SCH_JQ = 1               # schraudolph only for q-tiles >= this (q >= 512)

_cache = {}
last_results = None


def _build_nc(zero_bias=True):
    nc = bacc.Bacc(None, target_bir_lowering=False)
    x_kT = nc.declare_dram_parameter("x_kT", [EMB, S], F16, isOutput=False)
    w_vT = nc.declare_dram_parameter("w_vT", [EMB, DPC], F16, isOutput=False)
    QKDT = F8 if FP8_PROJ else F16
    x_qT = nc.declare_dram_parameter("x_qT", [EMB, S], QKDT, isOutput=False)
    if FP8_PROJ:
        x_kT8 = nc.declare_dram_parameter("x_kT8", [EMB, S], QKDT, isOutput=False)
    w_qT = nc.declare_dram_parameter("w_qT", [EMB, DPC], QKDT, isOutput=False)
    w_kT = nc.declare_dram_parameter("w_kT", [EMB, DPC], QKDT, isOutput=False)
    b_qk = nc.declare_dram_parameter("b_qk", [128, 2 * ND], F32, isOutput=False)
    consts = nc.declare_dram_parameter("consts", [128, 4 * 1024], F16, isOutput=False)
    z_raw = nc.declare_dram_parameter("z_raw", [HPC, VW, S], F16, isOutput=True)

    with TileContext(nc) as tc:
        with tc.tile_pool(name="const", bufs=1) as cp, \
             tc.tile_pool(name="xk16", bufs=NQ) as xp16, \
             tc.tile_pool(name="x8", bufs=(2 * NQ if FP8_PROJ else NQ)) as xp8, \
             tc.tile_pool(name="pt", bufs=8) as pp, \
             tc.tile_pool(name="zout", bufs=2 * HPC) as zo:
            # persistent SBUF tensors
            wv_sb = cp.tile([128, NE * DPC], F16)
            wq_sb = cp.tile([128, NE * DPC], QKDT)
            wk_sb = cp.tile([128, NE * DPC], QKDT)
            bqk_sb = cp.tile([128, 2 * ND], F32)
            um_sb = cp.tile([128, 4 * 1024], F16)
            QT = cp.tile([128, ND * S], F16)     # [dim-in-dblk, dblk*S + tok]
            KT = cp.tile([128, ND * S], F16)
            VP = cp.tile([128, NT * HPC * VW], F16)  # [tok-in-blk, blk*520 + h*65 + d]

            bq_sb, bk_sb = bqk_sb[:, 0:ND], bqk_sb[:, ND:2 * ND]

            # warm tile memset first: no DMA deps, so the warmup matmuls can
            # run during the input load instead of queueing behind DMA waits
            warm = cp.tile([128, 512], F16)
            nc.vector.memset(warm[:, :], 0.25)

            # ---- DMAs in first-use order ----
            sxk, sxk8, sxq8 = [], [], []

            def dma_xk16(qb):
                t = xp16.tile([128, NE * 512], F16, tag="xk16", name=f"sxk{qb}")
                nc.sync.dma_start(
                    out=t.rearrange("p (e t) -> p e t", e=NE),
                    in_=x_kT[:, qb * 512:(qb + 1) * 512]
                    .rearrange("(e p) t -> p e t", p=128))
                sxk.append(t)

            def dma_x8(lst, src, qb, nm):
                t = xp8.tile([128, NE * 512], QKDT, tag="x8", name=f"{nm}{qb}")
                nc.sync.dma_start(
                    out=t.rearrange("p (e t) -> p e t", e=NE),
                    in_=src[:, qb * 512:(qb + 1) * 512]
                    .rearrange("(e p) t -> p e t", p=128))
                lst.append(t)

            # wv and the first x_k stripe land as 2-e chunks so the first
            # V-projection matmuls can start ~8us earlier (finer splits lose
            # to the ~0.6us per-DMA descriptor-issue cost on the Sync queue)
            t = xp16.tile([128, NE * 512], F16, tag="xk16", name="sxk0")
            for e in range(0, NE, 2):
                nc.sync.dma_start(
                    out=wv_sb.rearrange("p (e d) -> p e d", e=NE)[:, e:e + 2, :],
                    in_=w_vT[e * 128:(e + 2) * 128, :]
                    .rearrange("(e p) d -> p e d", p=128))
                nc.sync.dma_start(
                    out=t.rearrange("p (e t) -> p e t", e=NE)[:, e:e + 2, :],
                    in_=x_kT[e * 128:(e + 2) * 128, 0:512]
                    .rearrange("(e p) t -> p e t", p=128))
            sxk.append(t)
            # um/bqk early: the DVE pre-warm copies below wait on these, and
            # they gate the whole DVE queue (first V-proj casts included)
            nc.sync.dma_start(out=bqk_sb[:, :], in_=b_qk[:, :])
            nc.sync.dma_start(out=um_sb[:, :], in_=consts[:, :])
            if FP8_PROJ:
                dma_x8(sxk8, x_kT8, 0, "sxk8_")
            nc.sync.dma_start(
                out=wk_sb.rearrange("p (e d) -> p e d", e=NE),
                in_=w_kT.rearrange("(e p) d -> p e d", p=128))
            dma_x8(sxq8, x_qT, 0, "sxq8_")
            nc.sync.dma_start(
                out=wq_sb.rearrange("p (e d) -> p e d", e=NE),
                in_=w_qT.rearrange("(e p) d -> p e d", p=128))
            for qb in range(1, NQ):
                dma_xk16(qb)
                if FP8_PROJ:
                    dma_x8(sxk8, x_kT8, qb, "sxk8_")
                dma_x8(sxq8, x_qT, qb, "sxq8_")
            if not FP8_PROJ:
                sxk8 = sxk    # K projection reads the fp16 x_k stripes

            # ones columns for the denominator trick (V copies fill cols 0-63;
            # only col 64 of each head-block needs the 1.0 fill)
            nc.vector.memset(
                VP.rearrange("p (t w) -> p t w", w=VW)[:, :, V:VW], 1.0)
            # pre-warm DVE's vector clock on the const DMAs so later DVE ops
            # don't each carry DMA-sem waits (walrus wait-slot limits)
            scr = cp.tile([128, 2], F32)
            scrh = cp.tile([128, 1], F16)
            nc.vector.tensor_copy(scr[:, 0:1], bqk_sb[:, 0:1])
            nc.vector.tensor_copy(scrh[:, 0:1], um_sb[:, 0:1])
            # pre-warm PE's clock too (dummy weight loads): fused LW+MM pairs
            # have a ~2-slot combined sync-wait budget in walrus codegen, so
            # absorb the const-DMA and DVE deps before real matmuls start
            for ap in (wq_sb, wk_sb, wv_sb, um_sb, scrh):
                nc.tensor.ldweights(ap[0:64, 0:1])

            with tc.tile_pool(name="pj", bufs=2, space="PSUM") as pj:
                wps = pj.tile([128, 512], F32, tag="big", bufs=3, name="warmps")
                for _ in range(16):
                    nc.tensor.matmul(wps[:, :], lhsT=warm[:, 0:128],
                                     rhs=warm[:, :], start=True, stop=True,
                                     skip_group_check=True)

                # V[t, d] with ones column; feeds the attention stream
                def proj_v(tb):
                    qb, t = divmod(tb, 4)
                    ps = pj.tile([128, 512], F32, tag="big", bufs=3, name=f"pv{tb}")
                    for e in range(NE):
                        nc.tensor.matmul(
                            ps[:, :],
                            lhsT=sxk[qb][:, e * 512 + t * 128: e * 512 + (t + 1) * 128],
                            rhs=wv_sb[:, e * DPC:(e + 1) * DPC],
                            start=(e == 0), stop=(e == NE - 1))
                    dst = VP[:, tb * (HPC * VW):(tb + 1) * (HPC * VW)]
                    dst = dst.rearrange("p (h w) -> p h w", w=VW)[:, :, 0:V]
                    nc.vector.tensor_copy(
                        dst, ps[:, :].rearrange("p (h w) -> p h w", w=V))

                # K^T / Q^T chunk for one (dblk, qb)
                def proj_kq(which, dblk, qb):
                    wsb, bsb, OUT, sx = ((wk_sb, bk_sb, KT, sxk8) if which == "k"
                                         else (wq_sb, bq_sb, QT, sxq8))
                    ps = pj.tile([128, 512], F32, tag="big", bufs=3,
                                 name=f"p{which}{dblk}{qb}")
                    if FP8_PROJ:
                        w3 = wsb.rearrange("p (e d) -> p e d", e=NE)
                        x3 = sx[qb].rearrange("p (e t) -> p e t", e=NE)
                        for ep in range(NE // 2):
                            nc.tensor.matmul(
                                ps[:, :],
                                lhsT=w3[:, 2 * ep:2 * ep + 2,
                                        dblk * 128:(dblk + 1) * 128],
                                rhs=x3[:, 2 * ep:2 * ep + 2, :],
                                start=(ep == 0), stop=(ep == NE // 2 - 1),
                                perf_mode=mybir.MatmulPerfMode.DoubleRow)
                    else:
                        for e in range(NE):
                            nc.tensor.matmul(
                                ps[:, :],
                                lhsT=wsb[:, e * DPC + dblk * 128:
                                         e * DPC + (dblk + 1) * 128],
                                rhs=sx[qb][:, e * 512:(e + 1) * 512],
                                start=(e == 0), stop=(e == NE - 1))
                    dst = OUT[:, dblk * S + qb * 512: dblk * S + (qb + 1) * 512]
                    if zero_bias:
                        # ScalarE copy: frees DVE time and releases the PSUM
                        # slot sooner (DVE queue is the busier one)
                        nc.scalar.copy(dst, ps[:, :])
                    else:
                        nc.vector.tensor_scalar_add(dst, ps[:, :],
                                                    bsb[:, dblk:dblk + 1])

                # prologue: everything computable from the early DMAs
                # (xk stripe 0, wk, xq stripe 0, wq) - all dblks' qb=0
                # chunks, so the PE has ~28us of work while inputs stream in
                for tb in range(4):
                    proj_v(tb)
                for d in range(ND):
                    proj_kq("k", d, 0)
                    proj_kq("q", d, 0)

                # attention for head pair (2*dblk, 2*dblk+1); both heads'
                # scores land in ONE [128,1024] PSUM tile via two concurrent
                # row-tiled matmuls, so exp handles both heads in one instr
                # pending = (pts of one g, heads, zps, nkv, jq, last-g?)
                # issued one g later so exp/mask have a full iteration of
                # slack before PE consumes pts - carried across jq AND dblk
                # boundaries
                pending = [None]

                def issue_pv(pend):
                    cur, heads_, zps_, nkv_, jq_, last = pend
                    for pt, i in cur:
                        for hi in (0, 1):
                            nc.tensor.matmul(
                                zps_[hi][:, :],
                                lhsT=VP[:, i * (HPC * VW) + heads_[hi] * VW:
                                        i * (HPC * VW) + (heads_[hi] + 1) * VW],
                                rhs=pt[:, hi * 512:(hi + 1) * 512],
                                start=(i == 0), stop=(i == nkv_ - 1),
                                skip_group_check=True)
                    if last:
                        for hi in (0, 1):
                            zsb = zo.tile([VW, 512], F16, tag="zsb",
                                          name=f"zsb{heads_[hi]}_{jq_}")
                            nc.scalar.copy(zsb[:, :], zps_[hi][:, :])
                            nc.sync.dma_start(
                                out=z_raw[heads_[hi], :,
                                          jq_ * 512:(jq_ + 1) * 512],
                                in_=zsb[:, :])

                def attention_pair(dblk, feed):
                    heads = (2 * dblk, 2 * dblk + 1)
                    for jq in range(NQ):
                        nkv = 4 * (jq + 1)
                        qs = slice(dblk * S + jq * 512, dblk * S + (jq + 1) * 512)
                        zps = [pj.tile([VW, 512], F32, tag="zps", bufs=2,
                                       name=f"z{hi}_{jq % 2}") for hi in (0, 1)]
                        for g in range(nkv // 2):
                            for _ in range(2):
                                if feed:
                                    feed.pop(0)()
                            cur = []
                            for bs in range(2):
                                i = 2 * g + bs
                                kv = slice(dblk * S + i * 128,
                                           dblk * S + (i + 1) * 128)
                                sp = pj.tile([128, 1024], F32, tag="big",
                                             bufs=3, name=f"sp{bs}")
                                nc.tensor.matmul(
                                    sp[:, 0:512], lhsT=KT[0:64, kv],
                                    rhs=QT[0:64, qs], start=True, stop=True,
                                    tile_position=(0, 0))
                                nc.tensor.matmul(
                                    sp[:, 512:1024], lhsT=KT[64:128, kv],
                                    rhs=QT[64:128, qs], start=True, stop=True,
                                    tile_position=(64, 0))
                                pt = pp.tile([128, 1024], F16, tag="pt",
                                             name=f"pt{bs}")
                                # per-g 1:1 ACT/DVE split keeps both engines
                                # under PE's per-g budget in feed-less phases
                                use_dve = (jq >= SCH_JQ and bs == 1)
                                if use_dve:
                                    nc.vector.tensor_scalar(
                                        pt[:, :].bitcast(I16), sp[:, :],
                                        float(SCH_A * SCOMP),
                                        float(15 * 1024 - SCH_C), MULT, ADD)
                                else:
                                    nc.scalar.activation(
                                        pt[:, :], sp[:, :], EXP,
                                        scale=float(SCOMP))
                                if i >= 4 * jq:      # diagonal: zero the
                                    bb = i - 4 * jq  # upper triangle on DVE
                                    nc.vector.tensor_mul(
                                        pt[:, :], pt[:, :],
                                        um_sb[:, bb * 1024:(bb + 1) * 1024])
                                cur.append((pt, i))
                            if pending[0]:
                                issue_pv(pending[0])
                            pending[0] = (cur, heads, zps, nkv, jq,
                                          g == nkv // 2 - 1)

                for dblk in range(ND):
                    feed = []
                    if dblk == 0:
                        for q in (1, 2, 3):
                            feed += [lambda q=q: proj_kq("k", 0, q),
                                     lambda q=q: proj_kq("q", 0, q)]
                            feed += [(lambda tb=tb: proj_v(tb))
                                     for tb in range(4 * q, 4 * q + 4)]
                    if dblk + 1 < ND:
                        feed += [(lambda w=w, d=dblk + 1, q=q: proj_kq(w, d, q))
                                 for q in range(1, NQ) for w in ("k", "q")]
                    attention_pair(dblk, feed)
                    for f in feed:
                        f()
                issue_pv(pending[0])

    nc.compile()
    return nc


def kernel(x_q, x_k_v, attn_mask, w_q, b_q, w_k, b_k, w_v, b_v):
    global last_results
    x_q = np.ascontiguousarray(x_q, np.float32)
    x_k_v = np.ascontiguousarray(x_k_v, np.float32)
    w_q, w_k, w_v = (np.asarray(a, np.float32) for a in (w_q, w_k, w_v))
    b_q, b_k, b_v = (np.asarray(a, np.float32) for a in (b_q, b_k, b_v))

    zero_bias = not (np.any(b_q) or np.any(b_k))
    key = f"nc{zero_bias}"
    if key not in _cache:
        _cache[key] = _build_nc(zero_bias)
    nc = _cache[key]

    scale = 1.0 / np.sqrt(np.float32(QK))
    qkdt = _F8NP if FP8_PROJ else np.float16
    wsc = W8SCALE if FP8_PROJ else 1.0
    xkT16 = [np.ascontiguousarray(x_k_v[b].T).astype(np.float16) for b in range(B)]
    xqT8 = [np.ascontiguousarray(x_q[b].T).astype(qkdt) for b in range(B)]
    xkT8 = ([np.ascontiguousarray(x_k_v[b].T).astype(qkdt) for b in range(B)]
            if FP8_PROJ else None)
    wqT = [np.ascontiguousarray((w_q[g * DPC:(g + 1) * DPC] * (scale * wsc)).T)
           .astype(qkdt) for g in range(2)]
    wkT = [np.ascontiguousarray((w_k[g * DPC:(g + 1) * DPC] * wsc).T)
           .astype(qkdt) for g in range(2)]
    wvT = [np.ascontiguousarray(w_v[g * DPC:(g + 1) * DPC].T).astype(np.float16)
           for g in range(2)]
    bq2 = [np.ascontiguousarray(
        (b_q[g * DPC:(g + 1) * DPC] * (scale * wsc * wsc)).reshape(ND, 128).T)
        for g in range(2)]
    bk2 = [np.ascontiguousarray(
        (b_k[g * DPC:(g + 1) * DPC] * (wsc * wsc)).reshape(ND, 128).T)
        for g in range(2)]
    # 0/1 causal keep-masks for the 4 diagonal 128x512 blocks, duplicated for
    # the two heads packed side-by-side in one [128,1024] probability tile
    p = np.arange(128)[:, None]
    qq = np.arange(512)[None, :]
    um = np.concatenate(
        [np.tile(np.where(128 * bb + p > qq, np.float32(0.0), np.float32(1.0)),
                 (1, 2))
         for bb in range(4)], axis=1).astype(np.float16)
    bqk2 = [np.ascontiguousarray(np.concatenate([bq2[g], bk2[g]], axis=1))
            for g in range(2)]

    in_maps = []
    for c in range(NCORE):
        b, g = c // 2, c % 2
        m = {
            "x_qT": xqT8[b], "x_kT": xkT16[b],
            "w_qT": wqT[g], "w_kT": wkT[g], "w_vT": wvT[g],
            "b_qk": bqk2[g], "consts": um,
        }
        if FP8_PROJ:
            m["x_kT8"] = xkT8[b]
        in_maps.append(m)

    trace = os.environ.get("KERNEL_TRACE", "") == "1"
    res = run_bass_kernel_spmd(nc, in_maps, list(range(NCORE)), trace=trace)
    last_results = res

    out = np.empty((B, S, H * V), np.float32)
    for c in range(NCORE):
        b, g = c // 2, c % 2
        zr = res.results[c]["z_raw"].astype(np.float32)   # [HPC, VW, S]
        z = zr[:, :V, :] / zr[:, V:VW, :]                  # [HPC, V, S]
        out[b, :, g * DPC:(g + 1) * DPC] = z.transpose(2, 0, 1).reshape(S, DPC)
    out += b_v[None, None, :]
    return out


# revision 45
# speedup vs baseline: 1.0275x; 1.0034x over previous
"""Multi-head causal attention (B=4, S=2048, H=16, d=64, EMB=1024) on 8 trn2 cores.

Sharding: core c handles batch b = c // 2 and head-group g = c % 2
(8 of 16 heads), i.e. a 512-wide slice of the QKV projection dims.
HW exec ~251us (baseline 322.7us).

Device kernel (per core), fp16 matmuls with fp32 PSUM accumulation:
  - Q^T, K^T projections in [dims, tokens] layout; V in [tokens, dims]
    with a ones-column per head (softmax denominator rides along in the
    PV matmul as output row 64). fp8 was evaluated and rejected: e4m3
    noise on x and w puts the output at rel err 0.021 > the 2e-2 gate.
  - Scores computed transposed: S^T[kv, q]; the two heads of a dim-block
    land in ONE [128,1024] PSUM tile via two CONCURRENT row-tiled matmuls
    (tile_position (0,0)/(64,0), disjoint PSUM banks) - a pair costs
    ~227ns vs 431ns serial.
  - exp split across ScalarE and VectorE per g-iteration: bs0 block ->
    ACTIVATE(Exp); bs1 block (rows q>=512 only) -> Schraudolph fp16 exp
    in one DVE op: i16 = round-sat(s*1477.32 + 15316) bitcast to fp16
    (error contribution 2e-4 thanks to softmax common-mode cancellation).
  - Causal mask: DVE multiply by a 0/1 mask on diagonal blocks (mask
    duplicated host-side so one [128,1024] tensor_tensor covers both heads).
  - PV matmuls run one g-iteration behind their scores (pending queue
    carried across jq/dblk seams) so exp/mask never sit on PE's critical
    path; projection chunks are fed into the attention stream to fill
    exp-latency gaps; all (dblk, qb=0) chunks run in the prologue since
    they only need the first DMA stripes.
  - Startup: garbage-data warmup matmuls engage the HAM clock gate while
    inputs stream in; wv/xk0 arrive as 2-e chunks so V-projection starts
    ~8us in; z output is drained per (head, jq) via ScalarE copies.
Host: x transposes + fp16 casts, weight slicing/transpose (1/sqrt(d)
folded into w_q), final divide-by-denominator + head concat + b_v add.
"""

import os
import sys

import numpy as np

for _p in ("/opt/trn_rl_repo",):
    if _p not in sys.path:
        sys.path.insert(0, _p)

import concourse.bass as bass
import concourse.bacc as bacc
import concourse.mybir as mybir
from concourse.tile import TileContext
from concourse.bass_utils import run_bass_kernel_spmd

try:
    import ml_dtypes
    _F8NP = ml_dtypes.float8_e4m3fn
except Exception:  # pragma: no cover
    _F8NP = None

EMB, QK, V, H = 1024, 64, 64, 16
B, S = 4, 2048
NCORE = 8
HPC = H // 2            # heads per core
DPC = HPC * QK          # projection dims per core (512)
VW = V + 1              # V plus ones-column (65)
NE = EMB // 128         # 8 contraction blocks
ND = DPC // 128         # 4 dim blocks
NQ = S // 512           # 4 q tiles
NT = S // 128           # 16 kv/token blocks
F32 = mybir.dt.float32
F16 = mybir.dt.float16
I16 = mybir.dt.int16
F8 = mybir.dt.float8e4
EXP = mybir.ActivationFunctionType.Exp
MULT = mybir.AluOpType.mult
ADD = mybir.AluOpType.add

FP8_PROJ = False         # fp8e4 DoubleRow Q/K projections: rel err 0.021 >
                         # the 2e-2 gate (e4m3 noise on x and w) - disabled
W8SCALE = 128.0          # pre-scale on w_q/w_k before fp8 cast
SCOMP = 2.0 ** -14 if FP8_PROJ else 1.0   # score compensation (x128 * x128)
SCH_A = 1024.0 / np.log(2.0)              # fp16 schraudolph multiplier
SCH_C = 44.0                              # rel-err-balancing offset
SCH_JQ = 1               # schraudolph only for q-tiles >= this (q >= 512)

_cache = {}
last_results = None


def _build_nc(zero_bias=True):
    nc = bacc.Bacc(None, target_bir_lowering=False)
    x_kT = nc.declare_dram_parameter("x_kT", [EMB, S], F16, isOutput=False)
    w_vT = nc.declare_dram_parameter("w_vT", [EMB, DPC], F16, isOutput=False)
    QKDT = F8 if FP8_PROJ else F16
    x_qT = nc.declare_dram_parameter("x_qT", [EMB, S], QKDT, isOutput=False)
    if FP8_PROJ:
        x_kT8 = nc.declare_dram_parameter("x_kT8", [EMB, S], QKDT, isOutput=False)
    w_qT = nc.declare_dram_parameter("w_qT", [EMB, DPC], QKDT, isOutput=False)
    w_kT = nc.declare_dram_parameter("w_kT", [EMB, DPC], QKDT, isOutput=False)
    b_qk = nc.declare_dram_parameter("b_qk", [128, 2 * ND], F32, isOutput=False)
    consts = nc.declare_dram_parameter("consts", [128, 4 * 1024], F16, isOutput=False)
    z_raw = nc.declare_dram_parameter("z_raw", [HPC, VW, S], F16, isOutput=True)

    with TileContext(nc) as tc:
        with tc.tile_pool(name="const", bufs=1) as cp, \
             tc.tile_pool(name="xk16", bufs=NQ) as xp16, \
             tc.tile_pool(name="x8", bufs=(2 * NQ if FP8_PROJ else NQ)) as xp8, \
             tc.tile_pool(name="pt", bufs=8) as pp, \
             tc.tile_pool(name="zout", bufs=2 * HPC) as zo:
            # persistent SBUF tensors
            wv_sb = cp.tile([128, NE * DPC], F16)
            wq_sb = cp.tile([128, NE * DPC], QKDT)
            wk_sb = cp.tile([128, NE * DPC], QKDT)
            bqk_sb = cp.tile([128, 2 * ND], F32)
            um_sb = cp.tile([128, 4 * 1024], F16)
            QT = cp.tile([128, ND * S], F16)     # [dim-in-dblk, dblk*S + tok]
            KT = cp.tile([128, ND * S], F16)
            VP = cp.tile([128, NT * HPC * VW], F16)  # [tok-in-blk, blk*520 + h*65 + d]

            bq_sb, bk_sb = bqk_sb[:, 0:ND], bqk_sb[:, ND:2 * ND]

            # warm tile memset first: no DMA deps, so the warmup matmuls can
            # run during the input load instead of queueing behind DMA waits
            warm = cp.tile([128, 512], F16)
            nc.vector.memset(warm[:, :], 0.25)

            # ---- DMAs in first-use order ----
            sxk, sxk8, sxq8 = [], [], []

            def dma_xk16(qb):
                t = xp16.tile([128, NE * 512], F16, tag="xk16", name=f"sxk{qb}")
                nc.sync.dma_start(
                    out=t.rearrange("p (e t) -> p e t", e=NE),
                    in_=x_kT[:, qb * 512:(qb + 1) * 512]
                    .rearrange("(e p) t -> p e t", p=128))
                sxk.append(t)

            def dma_x8(lst, src, qb, nm):
                t = xp8.tile([128, NE * 512], QKDT, tag="x8", name=f"{nm}{qb}")
                nc.sync.dma_start(
                    out=t.rearrange("p (e t) -> p e t", e=NE),
                    in_=src[:, qb * 512:(qb + 1) * 512]
                    .rearrange("(e p) t -> p e t", p=128))
                lst.append(t)

            # wv and the first x_k stripe land as 2-e chunks so the first
            # V-projection matmuls can start ~8us earlier (finer splits lose
            # to the ~0.6us per-DMA descriptor-issue cost on the Sync queue)
            t = xp16.tile([128, NE * 512], F16, tag="xk16", name="sxk0")
            for e in range(0, NE, 2):
                nc.sync.dma_start(
                    out=wv_sb.rearrange("p (e d) -> p e d", e=NE)[:, e:e + 2, :],
                    in_=w_vT[e * 128:(e + 2) * 128, :]
                    .rearrange("(e p) d -> p e d", p=128))
                nc.sync.dma_start(
                    out=t.rearrange("p (e t) -> p e t", e=NE)[:, e:e + 2, :],
                    in_=x_kT[e * 128:(e + 2) * 128, 0:512]
                    .rearrange("(e p) t -> p e t", p=128))
            sxk.append(t)
            # um/bqk early: the DVE pre-warm copies below wait on these, and
            # they gate the whole DVE queue (first V-proj casts included)
            nc.sync.dma_start(out=bqk_sb[:, :], in_=b_qk[:, :])
            nc.sync.dma_start(out=um_sb[:, :], in_=consts[:, :])
            if FP8_PROJ:
                dma_x8(sxk8, x_kT8, 0, "sxk8_")
            nc.sync.dma_start(
                out=wk_sb.rearrange("p (e d) -> p e d", e=NE),
                in_=w_kT.rearrange("(e p) d -> p e d", p=128))
            dma_x8(sxq8, x_qT, 0, "sxq8_")
            nc.sync.dma_start(
                out=wq_sb.rearrange("p (e d) -> p e d", e=NE),
                in_=w_qT.rearrange("(e p) d -> p e d", p=128))
            for qb in range(1, NQ):
                dma_xk16(qb)
                if FP8_PROJ:
                    dma_x8(sxk8, x_kT8, qb, "sxk8_")
                dma_x8(sxq8, x_qT, qb, "sxq8_")
            if not FP8_PROJ:
                sxk8 = sxk    # K projection reads the fp16 x_k stripes

            # ones columns for the denominator trick (V copies fill cols 0-63;
            # only col 64 of each head-block needs the 1.0 fill)
            nc.vector.memset(
                VP.rearrange("p (t w) -> p t w", w=VW)[:, :, V:VW], 1.0)
            # pre-warm DVE's vector clock on the const DMAs so later DVE ops
            # don't each carry DMA-sem waits (walrus wait-slot limits)
            scr = cp.tile([128, 2], F32)
            scrh = cp.tile([128, 1], F16)
            nc.vector.tensor_copy(scr[:, 0:1], bqk_sb[:, 0:1])
            nc.vector.tensor_copy(scrh[:, 0:1], um_sb[:, 0:1])
            # pre-warm PE's clock too (dummy weight loads): fused LW+MM pairs
            # have a ~2-slot combined sync-wait budget in walrus codegen, so
            # absorb the const-DMA and DVE deps before real matmuls start
            for ap in (wq_sb, wk_sb, wv_sb, um_sb, scrh):
                nc.tensor.ldweights(ap[0:64, 0:1])

            with tc.tile_pool(name="pj", bufs=2, space="PSUM") as pj:
                wps = pj.tile([128, 512], F32, tag="big", bufs=3, name="warmps")
                for _ in range(16):
                    nc.tensor.matmul(wps[:, :], lhsT=warm[:, 0:128],
                                     rhs=warm[:, :], start=True, stop=True,
                                     skip_group_check=True)

                # V[t, d] with ones column; feeds the attention stream
                def proj_v(tb):
                    qb, t = divmod(tb, 4)
                    ps = pj.tile([128, 512], F32, tag="big", bufs=3, name=f"pv{tb}")
                    for e in range(NE):
                        nc.tensor.matmul(
                            ps[:, :],
                            lhsT=sxk[qb][:, e * 512 + t * 128: e * 512 + (t + 1) * 128],
                            rhs=wv_sb[:, e * DPC:(e + 1) * DPC],
                            start=(e == 0), stop=(e == NE - 1))
                    dst = VP[:, tb * (HPC * VW):(tb + 1) * (HPC * VW)]
                    dst = dst.rearrange("p (h w) -> p h w", w=VW)[:, :, 0:V]
                    nc.vector.tensor_copy(
                        dst, ps[:, :].rearrange("p (h w) -> p h w", w=V))

                # K^T / Q^T chunk for one (dblk, qb)
                def proj_kq(which, dblk, qb):
                    wsb, bsb, OUT, sx = ((wk_sb, bk_sb, KT, sxk8) if which == "k"
                                         else (wq_sb, bq_sb, QT, sxq8))
                    ps = pj.tile([128, 512], F32, tag="big", bufs=3,
                                 name=f"p{which}{dblk}{qb}")
                    if FP8_PROJ:
                        w3 = wsb.rearrange("p (e d) -> p e d", e=NE)
                        x3 = sx[qb].rearrange("p (e t) -> p e t", e=NE)
                        for ep in range(NE // 2):
                            nc.tensor.matmul(
                                ps[:, :],
                                lhsT=w3[:, 2 * ep:2 * ep + 2,
                                        dblk * 128:(dblk + 1) * 128],
                                rhs=x3[:, 2 * ep:2 * ep + 2, :],
                                start=(ep == 0), stop=(ep == NE // 2 - 1),
                                perf_mode=mybir.MatmulPerfMode.DoubleRow)
                    else:
                        for e in range(NE):
                            nc.tensor.matmul(
                                ps[:, :],
                                lhsT=wsb[:, e * DPC + dblk * 128:
                                         e * DPC + (dblk + 1) * 128],
                                rhs=sx[qb][:, e * 512:(e + 1) * 512],
                                start=(e == 0), stop=(e == NE - 1))
                    dst = OUT[:, dblk * S + qb * 512: dblk * S + (qb + 1) * 512]
                    if zero_bias:
                        # ScalarE copy: frees DVE time and releases the PSUM
                        # slot sooner (DVE queue is the busier one)
                        nc.scalar.copy(dst, ps[:, :])
                    else:
                        nc.vector.tensor_scalar_add(dst, ps[:, :],
                                                    bsb[:, dblk:dblk + 1])

                # prologue: everything computable from the early DMAs
                # (xk stripe 0, wk, xq stripe 0, wq) - all dblks' qb=0
                # chunks, so the PE has ~28us of work while inputs stream in
                for tb in range(4):
                    proj_v(tb)
                for d in range(ND):
                    proj_kq("k", d, 0)
                    proj_kq("q", d, 0)

                # attention for head pair (2*dblk, 2*dblk+1); both heads'
                # scores land in ONE [128,1024] PSUM tile via two concurrent
                # row-tiled matmuls, so exp handles both heads in one instr
                # pending = (pts of one g, heads, zps, nkv, jq, last-g?)
                # issued one g later so exp/mask have a full iteration of
                # slack before PE consumes pts - carried across jq AND dblk
                # boundaries
                pending = [None]

                def issue_pv(pend):
                    cur, heads_, zps_, nkv_, jq_, last = pend
                    for pt, i in cur:
                        for hi in (0, 1):
                            nc.tensor.matmul(
                                zps_[hi][:, :],
                                lhsT=VP[:, i * (HPC * VW) + heads_[hi] * VW:
                                        i * (HPC * VW) + (heads_[hi] + 1) * VW],
                                rhs=pt[:, hi * 512:(hi + 1) * 512],
                                start=(i == 0), stop=(i == nkv_ - 1),
                                skip_group_check=True)
                    if last:
                        for hi in (0, 1):
                            zsb = zo.tile([VW, 512], F16, tag="zsb",
                                          name=f"zsb{heads_[hi]}_{jq_}")
                            nc.scalar.copy(zsb[:, :], zps_[hi][:, :])
                            nc.sync.dma_start(
                                out=z_raw[heads_[hi], :,
                                          jq_ * 512:(jq_ + 1) * 512],
                                in_=zsb[:, :])

                def attention_pair(dblk, feed):
                    heads = (2 * dblk, 2 * dblk + 1)
                    for jq in range(NQ):
                        nkv = 4 * (jq + 1)
                        qs = slice(dblk * S + jq * 512, dblk * S + (jq + 1) * 512)
                        zps = [pj.tile([VW, 512], F32, tag="zps", bufs=2,
                                       name=f"z{hi}_{jq % 2}") for hi in (0, 1)]
                        for g in range(nkv // 2):
                            for _ in range(2):
                                if feed:
                                    feed.pop(0)()
                            cur = []
                            for bs in range(2):
                                i = 2 * g + bs
                                kv = slice(dblk * S + i * 128,
                                           dblk * S + (i + 1) * 128)
                                sp = pj.tile([128, 1024], F32, tag="big",
                                             bufs=3, name=f"sp{bs}")
                                nc.tensor.matmul(
                                    sp[:, 0:512], lhsT=KT[0:64, kv],
                                    rhs=QT[0:64, qs], start=True, stop=True,
                                    tile_position=(0, 0))
                                nc.tensor.matmul(
                                    sp[:, 512:1024], lhsT=KT[64:128, kv],
                                    rhs=QT[64:128, qs], start=True, stop=True,
                                    tile_position=(64, 0))
                                pt = pp.tile([128, 1024], F16, tag="pt",
                                             name=f"pt{bs}")
                                # per-g 1:1 ACT/DVE split keeps both engines
                                # under PE's per-g budget in feed-less phases
                                use_dve = (jq >= SCH_JQ and bs == 1)
                                if use_dve:
                                    nc.vector.tensor_scalar(
                                        pt[:, :].bitcast(I16), sp[:, :],
                                        float(SCH_A * SCOMP),
                                        float(15 * 1024 - SCH_C), MULT, ADD)
                                else:
                                    nc.scalar.activation(
                                        pt[:, :], sp[:, :], EXP,
                                        scale=float(SCOMP))
                                if i >= 4 * jq:      # diagonal: zero the
                                    bb = i - 4 * jq  # upper triangle on DVE
                                    nc.vector.tensor_mul(
                                        pt[:, :], pt[:, :],
                                        um_sb[:, bb * 1024:(bb + 1) * 1024])
                                cur.append((pt, i))
                            if pending[0]:
                                issue_pv(pending[0])
                            pending[0] = (cur, heads, zps, nkv, jq,
                                          g == nkv // 2 - 1)

                for dblk in range(ND):
                    feed = []
                    if dblk == 0:
                        for q in (1, 2, 3):
                            feed += [lambda q=q: proj_kq("k", 0, q),
                                     lambda q=q: proj_kq("q", 0, q)]
                            feed += [(lambda tb=tb: proj_v(tb))
                                     for tb in range(4 * q, 4 * q + 4)]
                    if dblk + 1 < ND:
                        feed += [(lambda w=w, d=dblk + 1, q=q: proj_kq(w, d, q))
                                 for q in range(1, NQ) for w in ("k", "q")]
                    attention_pair(dblk, feed)
                    for f in feed:
                        f()
                issue_pv(pending[0])

    nc.compile()
    return nc


def kernel(x_q, x_k_v, attn_mask, w_q, b_q, w_k, b_k, w_v, b_v):
    global last_results
    x_q = np.ascontiguousarray(x_q, np.float32)
    x_k_v = np.ascontiguousarray(x_k_v, np.float32)
    w_q, w_k, w_v = (np.asarray(a, np.float32) for a in (w_q, w_k, w_v))
    b_q, b_k, b_v = (np.asarray(a, np.float32) for a in (b_q, b_k, b_v))

    zero_bias = not (np.any(b_q) or np.any(b_k))
    key = f"nc{zero_bias}"
    if key not in _cache:
        _cache[key] = _build_nc(zero_bias)
    nc = _cache[key]

    scale = 1.0 / np.sqrt(np.float32(QK))
    qkdt = _F8NP if FP8_PROJ else np.float16
    wsc = W8SCALE if FP8_PROJ else 1.0
    xkT16 = [np.ascontiguousarray(x_k_v[b].T).astype(np.float16) for b in range(B)]
    xqT8 = [np.ascontiguousarray(x_q[b].T).astype(qkdt) for b in range(B)]
    xkT8 = ([np.ascontiguousarray(x_k_v[b].T).astype(qkdt) for b in range(B)]
            if FP8_PROJ else None)
    wqT = [np.ascontiguousarray((w_q[g * DPC:(g + 1) * DPC] * (scale * wsc)).T)
           .astype(qkdt) for g in range(2)]
    wkT = [np.ascontiguousarray((w_k[g * DPC:(g + 1) * DPC] * wsc).T)
           .astype(qkdt) for g in range(2)]
    wvT = [np.ascontiguousarray(w_v[g * DPC:(g + 1) * DPC].T).astype(np.float16)
           for g in range(2)]
    bq2 = [np.ascontiguousarray(
        (b_q[g * DPC:(g + 1) * DPC] * (scale * wsc * wsc)).reshape(ND, 128).T)
        for g in range(2)]
    bk2 = [np.ascontiguousarray(
        (b_k[g * DPC:(g + 1) * DPC] * (wsc * wsc)).reshape(ND, 128).T)
        for g in range(2)]
    # 0/1 causal keep-masks for the 4 diagonal 128x512 blocks, duplicated for
    # the two heads packed side-by-side in one [128,1024] probability tile
    p = np.arange(128)[:, None]
    qq = np.arange(512)[None, :]
    um = np.concatenate(
        [np.tile(np.where(128 * bb + p > qq, np.float32(0.0), np.float32(1.0)),
                 (1, 2))
         for bb in range(4)], axis=1).astype(np.float16)
    bqk2 = [np.ascontiguousarray(np.concatenate([bq2[g], bk2[g]], axis=1))
            for g in range(2)]

    in_maps = []
    for c in range(NCORE):
        b, g = c // 2, c % 2
        m = {
            "x_qT": xqT8[b], "x_kT": xkT16[b],
            "w_qT": wqT[g], "w_kT": wkT[g], "w_vT": wvT[g],
            "b_qk": bqk2[g], "consts": um,
        }
        if FP8_PROJ:
            m["x_kT8"] = xkT8[b]
        in_maps.append(m)

    trace = os.environ.get("KERNEL_TRACE", "") == "1"
    res = run_bass_kernel_spmd(nc, in_maps, list(range(NCORE)), trace=trace)
    last_results = res

    out = np.empty((B, S, H * V), np.float32)
    for c in range(NCORE):
        b, g = c // 2, c % 2
        zr = res.results[c]["z_raw"].astype(np.float32)   # [HPC, VW, S]
        z = zr[:, :V, :] / zr[:, V:VW, :]                  # [HPC, V, S]
        out[b, :, g * DPC:(g + 1) * DPC] = z.transpose(2, 0, 1).reshape(S, DPC)
    out += b_v[None, None, :]
    return out
